# revision 3
# baseline (speedup 1.0000x reference)
"""Trainium2 Bass kernel for nn_ATTEfficient (ragged segment attention pooling).

reference:
    H = tanh(features @ Ww.T + bw)          # [TOTAL, D]
    s = H @ v                                # [TOTAL]
    att = segment_softmax(s, segment_ids)    # [TOTAL]
    pooled = segment_sum(features * att)     # [N_SEG, D]
    h = relu(pooled @ W1.T + b1)             # [N_SEG, D_HEAD]
    out = h @ W2.T + b2                      # [N_SEG, 1]

Sharding: segments (and their contiguous token ranges) are split across the
8 cores data-parallel, balanced by token count; weights replicated; all
segment reductions are local to one core.

Device pipeline per core (single fused loop over 512-token tiles):
    - H.T tile = tanh(WwT_blk.T @ XT_slab + bw) on PE (float32r) + ACT
    - s column blocks via lhsT=H.T chunk, rhs=v  -> [128,4] psum per tile
    - e = exp(s) (ACT, no max subtraction: |s| is O(1) for this model)
    - A = onehot_mask * e (DVE), pooling psum += A.T @ X_natural (PE)
    - z psum += A.T @ ones
  tail: pooled = pooled_raw / z, head matmuls + relu + final dot on PE.
"""

import os
import numpy as np

import concourse.bass as bass
import concourse.tile as tile
from concourse import bacc, mybir
from concourse.bass_utils import run_bass_kernel_spmd
from concourse.masks import make_identity

F32 = mybir.dt.float32
F32R = mybir.dt.float32r
AF = mybir.ActivationFunctionType

N_CORES = 8
N_SEG = 128
D = 1280
KB = D // 128  # 10 k/m blocks of the 1280 feature dim
DH = 512
HB = DH // 128  # 4 head blocks

LAST_RESULTS = None  # BassKernelResults of the most recent run (for test.py)
_PROGRAM_CACHE = {}


def _partition_segments(lengths: np.ndarray) -> list[int]:
    """Split N_SEG contiguous segments into N_CORES contiguous groups with
    roughly equal token counts. Returns cut indices, len N_CORES+1."""
    prefix = np.cumsum(lengths)  # tokens through segment i (inclusive)
    total = int(prefix[-1])
    cuts = [0]
    for c in range(1, N_CORES):
        target = total * c / N_CORES
        k = int(np.searchsorted(prefix, target))  # first idx with prefix >= target
        # boundary after segment k-1 vs after segment k: pick nearer to target
        cand = []
        for kk in (k, k + 1):
            if cuts[-1] + 1 <= kk <= N_SEG - (N_CORES - c):
                tok = prefix[kk - 1] if kk >= 1 else 0
                cand.append((abs(tok - target), kk))
        if not cand:
            kk = min(max(cuts[-1] + 1, k), N_SEG - (N_CORES - c))
            cand.append((0, kk))
        cuts.append(min(cand)[1])
    cuts.append(N_SEG)
    return cuts


def _emit(tc: tile.TileContext, t: dict, T_pad: int, S: int):
    nc = tc.nc
    NT = T_pad // 512
    NB = T_pad // 128

    with tc.tile_pool(name="const", bufs=1) as cp:
        # resident weights / masks
        wwt_sb = cp.tile([128, KB, D], F32R)
        nc.sync.dma_start(out=wwt_sb[:], in_=t["wwt"].rearrange("(kb p) m -> p kb m", p=128))
        bwd_sb = cp.tile([128, KB], F32)
        nc.sync.dma_start(out=bwd_sb[:], in_=t["bwd"][:])
        vd_sb = cp.tile([128, KB, 2], F32R)
        nc.sync.dma_start(out=vd_sb[:], in_=t["vd"].rearrange("p (kb two) -> p kb two", two=2))
        m1h_sb = cp.tile([128, NB * S], F32)
        nc.sync.dma_start(out=m1h_sb[:], in_=t["m1h"][:])

        zo_sb = cp.tile([128, 514], F32R)
        nc.sync.dma_start(out=zo_sb[:], in_=t["zo"][:])

        eP_sb = cp.tile([128, NB * 2], F32)
        A_sb = cp.tile([128, NB * S], F32R)

        with tc.tile_pool(name="accps", bufs=1, space="PSUM") as accp:
            pooled_ps = accp.tile([S, 1536], F32)  # 3 banks, cols 0:D used
            z_ps = accp.tile([S, 2], F32)          # 1 bank

            # zero-matmuls: make the accumulators order-safe (write whole
            # used range once with start=True; accumulating matmuls are all
            # start=False and commute)
            for c0 in range(0, D, 512):
                cw = min(512, D - c0)
                nc.tensor.matmul(pooled_ps[:, c0:c0 + cw], zo_sb[:, 0:S],
                                 zo_sb[:, 0:cw], start=True, stop=False,
                                 skip_group_check=True)
            nc.tensor.matmul(z_ps[:], zo_sb[:, 0:S], zo_sb[:, 0:2],
                             start=True, stop=False, skip_group_check=True)

            with tc.tile_pool(name="xtp", bufs=2) as xtp, \
                 tc.tile_pool(name="htp", bufs=3) as htp, \
                 tc.tile_pool(name="xnp", bufs=4) as xnp, \
                 tc.tile_pool(name="hps", bufs=2, space="PSUM") as hpsp, \
                 tc.tile_pool(name="sps", bufs=2, space="PSUM") as spsp:
                for nt in range(NT):
                    xt_sb = xtp.tile([128, KB, 512], F32R)
                    nc.sync.dma_start(
                        out=xt_sb[:],
                        in_=t["xt"][:, nt * 512:(nt + 1) * 512]
                            .rearrange("(kb p) n -> p kb n", p=128))

                    s_ps = spsp.tile([128, 8], F32)
                    nc.tensor.matmul(s_ps[:], zo_sb[:, 0:128],
                                     zo_sb[:, 0:8], start=True, stop=False,
                                     skip_group_check=True)

                    for mb in range(KB):
                        h_ps = hpsp.tile([128, 512], F32)
                        for kb in range(KB):
                            nc.tensor.matmul(
                                h_ps[:],
                                wwt_sb[:, kb, mb * 128:(mb + 1) * 128],
                                xt_sb[:, kb, :],
                                start=(kb == 0), stop=(kb == KB - 1))
                        ht_sb = htp.tile([128, 512], F32R)
                        nc.scalar.activation(ht_sb[:], h_ps[:], AF.Tanh,
                                             bias=bwd_sb[:, mb:mb + 1])
                        for j in range(4):
                            nc.tensor.matmul(
                                s_ps[:, 2 * j:2 * j + 2],
                                ht_sb[:, j * 128:(j + 1) * 128],
                                vd_sb[:, mb, :],
                                start=False,
                                stop=(mb == KB - 1 and j == 3),
                                skip_group_check=True)

                    # e = exp(s) for this tile's 4 token blocks
                    nc.scalar.activation(eP_sb[:, nt * 8:nt * 8 + 8], s_ps[:], AF.Exp)

                    # pooling for the 4 blocks of this tile
                    for bj in range(4):
                        b = nt * 4 + bj
                        nc.vector.tensor_scalar_mul(
                            A_sb[:, b * S:(b + 1) * S],
                            m1h_sb[:, b * S:(b + 1) * S],
                            eP_sb[:, 2 * b:2 * b + 1])
                        xn_sb = xnp.tile([128, D], F32R)
                        nc.sync.dma_start(out=xn_sb[:],
                                          in_=t["xn"][b * 128:(b + 1) * 128, :])
                        for c0 in range(0, D, 512):
                            cw = min(512, D - c0)
                            nc.tensor.matmul(
                                pooled_ps[:, c0:c0 + cw],
                                A_sb[:, b * S:(b + 1) * S],
                                xn_sb[:, c0:c0 + cw],
                                start=False,
                                stop=(b == NB - 1 and c0 + cw == D),
                                skip_group_check=True)
                        nc.tensor.matmul(
                            z_ps[:], A_sb[:, b * S:(b + 1) * S], zo_sb[:, 512:514],
                            start=False, stop=(b == NB - 1),
                            skip_group_check=True)

            # ---- head ----
            w1t_sb = cp.tile([128, KB, DH], F32R)
            nc.sync.dma_start(out=w1t_sb[:],
                              in_=t["w1t"].rearrange("(kb p) m -> p kb m", p=128))
            b1d_sb = cp.tile([128, HB], F32)
            nc.sync.dma_start(out=b1d_sb[:], in_=t["b1d"][:])
            w2d_sb = cp.tile([128, HB], F32R)
            nc.sync.dma_start(out=w2d_sb[:], in_=t["w2d"][:])
            b2d_sb = cp.tile([1, 1], F32)
            nc.sync.dma_start(out=b2d_sb[:], in_=t["b2d"][:])
            identS = cp.tile([S, S], F32)
            make_identity(nc, identS[:])

            zc_sb = cp.tile([S, 1], F32)
            nc.vector.tensor_scalar_max(zc_sb[:], z_ps[:, 0:1], 1e-30)
            rz_sb = cp.tile([S, 1], F32)
            nc.vector.reciprocal(rz_sb[:], zc_sb[:])
            pooled_sb = cp.tile([S, D], F32)
            nc.vector.tensor_scalar_mul(pooled_sb[:], pooled_ps[:, 0:D], rz_sb[:])

        with tc.tile_pool(name="p3ps", bufs=2, space="PSUM") as p3p:
            pT_sb = cp.tile([128, KB * S], F32R)
            for db in range(KB):
                pT_ps = p3p.tile([128, S], F32)
                nc.tensor.transpose(pT_ps[:], pooled_sb[:, db * 128:(db + 1) * 128],
                                    identS[:])
                nc.vector.tensor_copy(pT_sb[:, db * S:(db + 1) * S], pT_ps[:])

            h3_sb = cp.tile([128, HB * S], F32R)
            for hb in range(HB):
                h3_ps = p3p.tile([128, S], F32)
                for db in range(KB):
                    nc.tensor.matmul(
                        h3_ps[:],
                        w1t_sb[:, db, hb * 128:(hb + 1) * 128],
                        pT_sb[:, db * S:(db + 1) * S],
                        start=(db == 0), stop=(db == KB - 1))
                nc.scalar.activation(h3_sb[:, hb * S:(hb + 1) * S], h3_ps[:],
                                     AF.Relu, bias=b1d_sb[:, hb:hb + 1])

            o_ps = p3p.tile([1, S], F32)
            for hb in range(HB):
                nc.tensor.matmul(o_ps[:], w2d_sb[:, hb:hb + 1],
                                 h3_sb[:, hb * S:(hb + 1) * S],
                                 start=(hb == 0), stop=(hb == HB - 1))
            out_sb = cp.tile([1, S], F32)
            nc.scalar.activation(out_sb[:], o_ps[:], AF.Identity,
                                 bias=b2d_sb[0:1, 0:1])
            nc.sync.dma_start(out=t["out"][:], in_=out_sb[:])


def _build_program(T_pad: int, S: int):
    key = (T_pad, S)
    if key in _PROGRAM_CACHE:
        return _PROGRAM_CACHE[key]
    NB = T_pad // 128
    nc = bacc.Bacc("TRN2", target_bir_lowering=False, debug=False,
                   num_devices=N_CORES)
    t = {
        "xt": nc.dram_tensor("xt", [D, T_pad], F32R, kind="ExternalInput").ap(),
        "xn": nc.dram_tensor("xn", [T_pad, D], F32R, kind="ExternalInput").ap(),
        "wwt": nc.dram_tensor("wwt", [D, D], F32R, kind="ExternalInput").ap(),
        "m1h": nc.dram_tensor("m1h", [128, NB * S], F32, kind="ExternalInput").ap(),
        "bwd": nc.dram_tensor("bwd", [128, KB], F32, kind="ExternalInput").ap(),
        "vd": nc.dram_tensor("vd", [128, KB * 2], F32R, kind="ExternalInput").ap(),
        "zo": nc.dram_tensor("zo", [128, 514], F32R, kind="ExternalInput").ap(),
        "w1t": nc.dram_tensor("w1t", [D, DH], F32R, kind="ExternalInput").ap(),
        "b1d": nc.dram_tensor("b1d", [128, HB], F32, kind="ExternalInput").ap(),
        "w2d": nc.dram_tensor("w2d", [128, HB], F32R, kind="ExternalInput").ap(),
        "b2d": nc.dram_tensor("b2d", [1, 1], F32, kind="ExternalInput").ap(),
        "out": nc.dram_tensor("out", [1, S], F32, kind="ExternalOutput").ap(),
    }
    with tile.TileContext(nc) as tc:
        _emit(tc, t, T_pad, S)
    nc.compile()
    _PROGRAM_CACHE[key] = nc
    return nc


def kernel(features, Ww, bw, v, W1, b1, W2, b2, segment_ids):
    global LAST_RESULTS
    features = np.ascontiguousarray(np.asarray(features, dtype=np.float32))
    Ww = np.asarray(Ww, dtype=np.float32)
    bw = np.asarray(bw, dtype=np.float32)
    v = np.asarray(v, dtype=np.float32)
    W1 = np.asarray(W1, dtype=np.float32)
    b1 = np.asarray(b1, dtype=np.float32)
    W2 = np.asarray(W2, dtype=np.float32)
    b2 = np.asarray(b2, dtype=np.float32)
    segment_ids = np.asarray(segment_ids)

    lengths = np.bincount(segment_ids.astype(np.int64), minlength=N_SEG)
    cuts = _partition_segments(lengths)
    seg_prefix = np.concatenate([[0], np.cumsum(lengths)])
    tok_cuts = [int(seg_prefix[c]) for c in cuts]

    S = max(cuts[c + 1] - cuts[c] for c in range(N_CORES))
    S += S % 2  # fp32r moving-operand count must be even
    T_max = max(tok_cuts[c + 1] - tok_cuts[c] for c in range(N_CORES))
    T_pad = max(512, ((T_max + 511) // 512) * 512)
    NB = T_pad // 128

    # shared (replicated) weight layouts
    wwt = np.ascontiguousarray(Ww.T)                    # [k, m]
    bwd = np.ascontiguousarray(bw.reshape(KB, 128).T)   # [128, KB]
    vd = np.zeros((128, KB, 2), dtype=np.float32)       # [128, KB, 2] (v, 0)
    vd[:, :, 0] = v.reshape(KB, 128).T
    vd = vd.reshape(128, KB * 2)
    zo = np.zeros((128, 514), dtype=np.float32)
    zo[:, 512:514] = 1.0
    w1t = np.ascontiguousarray(W1.T)                    # [k, h]
    b1d = np.ascontiguousarray(b1.reshape(HB, 128).T)   # [128, HB]
    w2d = np.ascontiguousarray(W2[0].reshape(HB, 128).T)
    b2d = b2.reshape(1, 1)

    in_maps = []
    for c in range(N_CORES):
        s0, s1 = cuts[c], cuts[c + 1]
        t0, t1 = tok_cuts[c], tok_cuts[c + 1]
        Tc = t1 - t0
        xn = np.zeros((T_pad, D), dtype=np.float32)
        xn[:Tc] = features[t0:t1]
        xt = np.ascontiguousarray(xn.T)
        oh = np.zeros((T_pad, S), dtype=np.float32)
        if Tc > 0:
            oh[np.arange(Tc), segment_ids[t0:t1].astype(np.int64) - s0] = 1.0
        m1h = np.ascontiguousarray(
            oh.reshape(NB, 128, S).transpose(1, 0, 2).reshape(128, NB * S))
        in_maps.append({
            "xt": xt, "xn": xn, "m1h": m1h,
            "wwt": wwt, "bwd": bwd, "vd": vd, "zo": zo,
            "w1t": w1t, "b1d": b1d, "w2d": w2d, "b2d": b2d,
        })

    nc = _build_program(T_pad, S)
    trace = bool(int(os.environ.get("KERNEL_TRACE", "0")))
    res = run_bass_kernel_spmd(nc, in_maps, core_ids=list(range(N_CORES)),
                               trace=trace)
    LAST_RESULTS = res

    out = np.zeros((N_SEG, 1), dtype=np.float32)
    for c in range(N_CORES):
        s0, s1 = cuts[c], cuts[c + 1]
        out[s0:s1, 0] = res.results[c]["out"][0, :s1 - s0]

    # empty segments: pooled = 0 -> out = relu(b1) @ W2.T + b2 (host patch;
    # device row may be NaN from 0 * (1/0))
    empty = lengths == 0
    if empty.any():
        out[empty, 0] = float(np.maximum(b1, 0.0) @ W2[0] + b2[0])
    return out


# revision 4
# speedup vs baseline: 1.0194x; 1.0194x over previous
"""Trainium2 Bass kernel for nn_ATTEfficient (ragged segment attention pooling).

reference:
    H = tanh(features @ Ww.T + bw)          # [TOTAL, D]
    s = H @ v                                # [TOTAL]
    att = segment_softmax(s, segment_ids)    # [TOTAL]
    pooled = segment_sum(features * att)     # [N_SEG, D]
    h = relu(pooled @ W1.T + b1)             # [N_SEG, D_HEAD]
    out = h @ W2.T + b2                      # [N_SEG, 1]

Sharding: segments (and their contiguous token ranges) are split across the
8 cores data-parallel, balanced by token count; weights replicated; all
segment reductions are local to one core.

Device pipeline per core (single fused loop over 512-token tiles):
    - H.T tile = tanh(WwT_blk.T @ XT_slab + bw) on PE (float32r) + ACT
    - s column blocks via lhsT=H.T chunk, rhs=v  -> [128,4] psum per tile
    - e = exp(s) (ACT, no max subtraction: |s| is O(1) for this model)
    - A = onehot_mask * e (DVE), pooling psum += A.T @ X_natural (PE)
    - z psum += A.T @ ones
  tail: pooled = pooled_raw / z, head matmuls + relu + final dot on PE.
"""

import os
import numpy as np

import concourse.bass as bass
import concourse.tile as tile
from concourse import bacc, mybir
from concourse.bass_utils import run_bass_kernel_spmd
from concourse.masks import make_identity

F32 = mybir.dt.float32
F32R = mybir.dt.float32r
AF = mybir.ActivationFunctionType

N_CORES = 8
N_SEG = 128
D = 1280
KB = D // 128  # 10 k/m blocks of the 1280 feature dim
DH = 512
HB = DH // 128  # 4 head blocks

LAST_RESULTS = None  # BassKernelResults of the most recent run (for test.py)
_PROGRAM_CACHE = {}


def _partition_segments(lengths: np.ndarray) -> list[int]:
    """Split N_SEG contiguous segments into N_CORES contiguous groups
    minimizing the max token count (binary search + greedy packing).
    Returns cut indices, len N_CORES+1."""
    lengths = lengths.astype(np.int64)
    total = int(lengths.sum())

    def cuts_for(cap):
        cuts = [0]
        cur = 0
        for i, L in enumerate(lengths):
            if cur + L > cap and cur > 0:
                cuts.append(i)
                cur = 0
                if len(cuts) > N_CORES:
                    return None
            cur += int(L)
        while len(cuts) < N_CORES:
            cuts.append(N_SEG)
        cuts.append(N_SEG)
        return cuts

    lo, hi = max(int(lengths.max()), (total + N_CORES - 1) // N_CORES), total
    while lo < hi:
        mid = (lo + hi) // 2
        if cuts_for(mid) is not None:
            hi = mid
        else:
            lo = mid + 1
    return cuts_for(lo)


def _emit(tc: tile.TileContext, t: dict, T_pad: int, S: int):
    nc = tc.nc
    NT = (T_pad + 511) // 512
    NB = T_pad // 128

    with tc.tile_pool(name="const", bufs=1) as cp:
        # resident weights / masks
        wwt_sb = cp.tile([128, KB, D], F32R)
        for k0, k1 in ((0, KB // 2), (KB // 2, KB)):
            nc.sync.dma_start(
                out=wwt_sb[:, k0:k1, :],
                in_=t["wwt"][k0 * 128:k1 * 128, :]
                    .rearrange("(kb p) m -> p kb m", p=128))
        bwd_sb = cp.tile([128, KB], F32)
        nc.sync.dma_start(out=bwd_sb[:], in_=t["bwd"][:])
        vd_sb = cp.tile([128, KB, 2], F32R)
        nc.sync.dma_start(out=vd_sb[:], in_=t["vd"].rearrange("p (kb two) -> p kb two", two=2))
        m1h_sb = cp.tile([128, NB * S], F32)
        nc.sync.dma_start(out=m1h_sb[:], in_=t["m1h"][:])

        zo_sb = cp.tile([128, 514], F32R)
        nc.sync.dma_start(out=zo_sb[:], in_=t["zo"][:])

        eP_sb = cp.tile([128, NB * 2], F32)
        A_sb = cp.tile([128, NB * S], F32R)

        w1t_sb = cp.tile([128, KB, DH], F32R)
        nc.sync.dma_start(out=w1t_sb[:],
                          in_=t["w1t"].rearrange("(kb p) m -> p kb m", p=128))
        b1d_sb = cp.tile([128, HB], F32)
        nc.sync.dma_start(out=b1d_sb[:], in_=t["b1d"][:])
        w2d_sb = cp.tile([128, HB], F32R)
        nc.sync.dma_start(out=w2d_sb[:], in_=t["w2d"][:])
        b2d_sb = cp.tile([1, 1], F32)
        nc.sync.dma_start(out=b2d_sb[:], in_=t["b2d"][:])
        identS = cp.tile([S, S], F32)
        make_identity(nc, identS[:])

        with tc.tile_pool(name="accps", bufs=1, space="PSUM") as accp:
            pooled_ps = accp.tile([S, 1536], F32)  # 3 banks; cols 0:D pooled, 1280:1282 z

            # zero-matmuls: make the accumulators order-safe (write whole
            # used range once with start=True; accumulating matmuls are all
            # start=False and commute)
            for c0, cw in ((0, 512), (512, 512), (1024, 258)):
                nc.tensor.matmul(pooled_ps[:, c0:c0 + cw], zo_sb[:, 0:S],
                                 zo_sb[:, 0:cw], start=True, stop=False,
                                 skip_group_check=True)

            with tc.tile_pool(name="xtp", bufs=2) as xtp, \
                 tc.tile_pool(name="htp", bufs=3) as htp, \
                 tc.tile_pool(name="xnp", bufs=4) as xnp, \
                 tc.tile_pool(name="hps", bufs=3, space="PSUM") as hpsp, \
                 tc.tile_pool(name="sps", bufs=2, space="PSUM") as spsp:
                for nt in range(NT):
                    W = min(512, T_pad - nt * 512)
                    xt_sb = xtp.tile([128, KB, 512], F32R)
                    for k0, k1 in ((0, KB // 2), (KB // 2, KB)):
                        nc.sync.dma_start(
                            out=xt_sb[:, k0:k1, 0:W],
                            in_=t["xt"][k0 * 128:k1 * 128, nt * 512:nt * 512 + W]
                                .rearrange("(kb p) n -> p kb n", p=128))

                    s_ps = spsp.tile([128, 8], F32)
                    nc.tensor.matmul(s_ps[:], zo_sb[:, 0:128],
                                     zo_sb[:, 0:8], start=True, stop=False,
                                     skip_group_check=True)

                    for mb in range(KB):
                        h_ps = hpsp.tile([128, 512], F32)
                        for kb in range(KB):
                            nc.tensor.matmul(
                                h_ps[:, 0:W],
                                wwt_sb[:, kb, mb * 128:(mb + 1) * 128],
                                xt_sb[:, kb, 0:W],
                                start=(kb == 0), stop=(kb == KB - 1))
                        ht_sb = htp.tile([128, 512], F32R)
                        nc.scalar.activation(ht_sb[:, 0:W], h_ps[:, 0:W], AF.Tanh,
                                             bias=bwd_sb[:, mb:mb + 1])
                        for j in range(W // 128):
                            nc.tensor.matmul(
                                s_ps[:, 2 * j:2 * j + 2],
                                ht_sb[:, j * 128:(j + 1) * 128],
                                vd_sb[:, mb, :],
                                start=False,
                                stop=(mb == KB - 1 and j == W // 128 - 1),
                                skip_group_check=True)

                    # e = exp(s) for this tile's token blocks
                    nc.scalar.activation(eP_sb[:, nt * 8:nt * 8 + 2 * (W // 128)],
                                         s_ps[:, 0:2 * (W // 128)], AF.Exp)

                    # pooling for the blocks of this tile
                    for bj in range(W // 128):
                        b = nt * 4 + bj
                        nc.vector.tensor_scalar_mul(
                            A_sb[:, b * S:(b + 1) * S],
                            m1h_sb[:, b * S:(b + 1) * S],
                            eP_sb[:, 2 * b:2 * b + 1])
                        xn_sb = xnp.tile([128, D], F32R)
                        nc.sync.dma_start(out=xn_sb[:],
                                          in_=t["xn"][b * 128:(b + 1) * 128, :])
                        for c0 in range(0, D, 512):
                            cw = min(512, D - c0)
                            nc.tensor.matmul(
                                pooled_ps[:, c0:c0 + cw],
                                A_sb[:, b * S:(b + 1) * S],
                                xn_sb[:, c0:c0 + cw],
                                start=False,
                                stop=(b == NB - 1 and c0 + cw == D),
                                skip_group_check=True)
                        nc.tensor.matmul(
                            pooled_ps[:, 1280:1282],
                            A_sb[:, b * S:(b + 1) * S], zo_sb[:, 512:514],
                            start=False, stop=(b == NB - 1),
                            skip_group_check=True)

            # ---- head ----
            zc_sb = cp.tile([S, 1], F32)
            nc.vector.tensor_scalar_max(zc_sb[:], pooled_ps[:, 1280:1281], 1e-30)
            rz_sb = cp.tile([S, 1], F32)
            nc.vector.reciprocal(rz_sb[:], zc_sb[:])
            pooled_sb = cp.tile([S, D], F32)
            nc.vector.tensor_scalar_mul(pooled_sb[:], pooled_ps[:, 0:D], rz_sb[:])

        with tc.tile_pool(name="p3ps", bufs=2, space="PSUM") as p3p:
            pT_sb = cp.tile([128, KB * S], F32R)
            for db in range(KB):
                pT_ps = p3p.tile([128, S], F32)
                nc.tensor.transpose(pT_ps[:], pooled_sb[:, db * 128:(db + 1) * 128],
                                    identS[:])
                nc.vector.tensor_copy(pT_sb[:, db * S:(db + 1) * S], pT_ps[:])

            h3_sb = cp.tile([128, HB * S], F32R)
            for hb in range(HB):
                h3_ps = p3p.tile([128, S], F32)
                for db in range(KB):
                    nc.tensor.matmul(
                        h3_ps[:],
                        w1t_sb[:, db, hb * 128:(hb + 1) * 128],
                        pT_sb[:, db * S:(db + 1) * S],
                        start=(db == 0), stop=(db == KB - 1))
                nc.scalar.activation(h3_sb[:, hb * S:(hb + 1) * S], h3_ps[:],
                                     AF.Relu, bias=b1d_sb[:, hb:hb + 1])

            o_ps = p3p.tile([1, S], F32)
            for hb in range(HB):
                nc.tensor.matmul(o_ps[:], w2d_sb[:, hb:hb + 1],
                                 h3_sb[:, hb * S:(hb + 1) * S],
                                 start=(hb == 0), stop=(hb == HB - 1))
            out_sb = cp.tile([1, S], F32)
            nc.scalar.activation(out_sb[:], o_ps[:], AF.Identity,
                                 bias=b2d_sb[0:1, 0:1])
            nc.sync.dma_start(out=t["out"][:], in_=out_sb[:])


def _build_program(T_pad: int, S: int):
    key = (T_pad, S)
    if key in _PROGRAM_CACHE:
        return _PROGRAM_CACHE[key]
    NB = T_pad // 128
    nc = bacc.Bacc("TRN2", target_bir_lowering=False, debug=False,
                   num_devices=N_CORES)
    t = {
        "xt": nc.dram_tensor("xt", [D, T_pad], F32R, kind="ExternalInput").ap(),
        "xn": nc.dram_tensor("xn", [T_pad, D], F32R, kind="ExternalInput").ap(),
        "wwt": nc.dram_tensor("wwt", [D, D], F32R, kind="ExternalInput").ap(),
        "m1h": nc.dram_tensor("m1h", [128, NB * S], F32, kind="ExternalInput").ap(),
        "bwd": nc.dram_tensor("bwd", [128, KB], F32, kind="ExternalInput").ap(),
        "vd": nc.dram_tensor("vd", [128, KB * 2], F32R, kind="ExternalInput").ap(),
        "zo": nc.dram_tensor("zo", [128, 514], F32R, kind="ExternalInput").ap(),
        "w1t": nc.dram_tensor("w1t", [D, DH], F32R, kind="ExternalInput").ap(),
        "b1d": nc.dram_tensor("b1d", [128, HB], F32, kind="ExternalInput").ap(),
        "w2d": nc.dram_tensor("w2d", [128, HB], F32R, kind="ExternalInput").ap(),
        "b2d": nc.dram_tensor("b2d", [1, 1], F32, kind="ExternalInput").ap(),
        "out": nc.dram_tensor("out", [1, S], F32, kind="ExternalOutput").ap(),
    }
    with tile.TileContext(nc) as tc:
        _emit(tc, t, T_pad, S)
    nc.compile()
    _PROGRAM_CACHE[key] = nc
    return nc


def kernel(features, Ww, bw, v, W1, b1, W2, b2, segment_ids):
    global LAST_RESULTS
    features = np.ascontiguousarray(np.asarray(features, dtype=np.float32))
    Ww = np.asarray(Ww, dtype=np.float32)
    bw = np.asarray(bw, dtype=np.float32)
    v = np.asarray(v, dtype=np.float32)
    W1 = np.asarray(W1, dtype=np.float32)
    b1 = np.asarray(b1, dtype=np.float32)
    W2 = np.asarray(W2, dtype=np.float32)
    b2 = np.asarray(b2, dtype=np.float32)
    segment_ids = np.asarray(segment_ids)

    lengths = np.bincount(segment_ids.astype(np.int64), minlength=N_SEG)
    cuts = _partition_segments(lengths)
    seg_prefix = np.concatenate([[0], np.cumsum(lengths)])
    tok_cuts = [int(seg_prefix[c]) for c in cuts]

    S = max(cuts[c + 1] - cuts[c] for c in range(N_CORES))
    S += S % 2  # fp32r moving-operand count must be even
    T_max = max(tok_cuts[c + 1] - tok_cuts[c] for c in range(N_CORES))
    T_pad = max(512, ((T_max + 127) // 128) * 128)
    NB = T_pad // 128

    # shared (replicated) weight layouts
    wwt = np.ascontiguousarray(Ww.T)                    # [k, m]
    bwd = np.ascontiguousarray(bw.reshape(KB, 128).T)   # [128, KB]
    vd = np.zeros((128, KB, 2), dtype=np.float32)       # [128, KB, 2] (v, 0)
    vd[:, :, 0] = v.reshape(KB, 128).T
    vd = vd.reshape(128, KB * 2)
    zo = np.zeros((128, 514), dtype=np.float32)
    zo[:, 512:514] = 1.0
    w1t = np.ascontiguousarray(W1.T)                    # [k, h]
    b1d = np.ascontiguousarray(b1.reshape(HB, 128).T)   # [128, HB]
    w2d = np.ascontiguousarray(W2[0].reshape(HB, 128).T)
    b2d = b2.reshape(1, 1)

    in_maps = []
    for c in range(N_CORES):
        s0, s1 = cuts[c], cuts[c + 1]
        t0, t1 = tok_cuts[c], tok_cuts[c + 1]
        Tc = t1 - t0
        xn = np.zeros((T_pad, D), dtype=np.float32)
        xn[:Tc] = features[t0:t1]
        xt = np.ascontiguousarray(xn.T)
        oh = np.zeros((T_pad, S), dtype=np.float32)
        if Tc > 0:
            oh[np.arange(Tc), segment_ids[t0:t1].astype(np.int64) - s0] = 1.0
        m1h = np.ascontiguousarray(
            oh.reshape(NB, 128, S).transpose(1, 0, 2).reshape(128, NB * S))
        in_maps.append({
            "xt": xt, "xn": xn, "m1h": m1h,
            "wwt": wwt, "bwd": bwd, "vd": vd, "zo": zo,
            "w1t": w1t, "b1d": b1d, "w2d": w2d, "b2d": b2d,
        })

    nc = _build_program(T_pad, S)
    trace = bool(int(os.environ.get("KERNEL_TRACE", "0")))
    res = run_bass_kernel_spmd(nc, in_maps, core_ids=list(range(N_CORES)),
                               trace=trace)
    LAST_RESULTS = res

    out = np.zeros((N_SEG, 1), dtype=np.float32)
    for c in range(N_CORES):
        s0, s1 = cuts[c], cuts[c + 1]
        out[s0:s1, 0] = res.results[c]["out"][0, :s1 - s0]

    # empty segments: pooled = 0 -> out = relu(b1) @ W2.T + b2 (host patch;
    # device row may be NaN from 0 * (1/0))
    empty = lengths == 0
    if empty.any():
        out[empty, 0] = float(np.maximum(b1, 0.0) @ W2[0] + b2[0])
    return out
